# revision 1
# baseline (speedup 1.0000x reference)
"""Trainium2 Bass kernel for a 2-layer GraphNetwork (gnn_message_passing).

Strategy:
  - 16 graphs are partitioned across 8 cores (2 graphs per core). Every
    edge's receiver node lives on the edge's own core, so ALL segment
    reductions (per-node edge means, per-graph pooling) are core-local.
    No collectives are needed; the [16,128] output rows are gathered on
    the host.
  - Per core, nodes are bin-packed into NT tiles of 128 "slots"; each
    tile's incoming edges are padded to K0 chunks of 128. Segment-sums
    are computed on the tensor engine as one-hot selector matmuls
    (the one-hot [128e x 128n] block is built on-device from a column
    index via DVE is_equal against an iota tile).
  - Weights are replicated; biases are folded into matmuls via ones-rows.
  - bf16 inputs/intermediates, fp32 PSUM accumulation, fp32 final stage.
"""

import numpy as np
import ml_dtypes

import concourse.bass as bass
import concourse.tile as tile_mod
from concourse import tile
from concourse.bass_utils import run_bass_kernel_spmd
from concourse.vector_clock import ScopedClock

mybir = bass.mybir

N_NODES, N_EDGES, N_GRAPHS = 20000, 320000, 16
F_NODE, F_EDGE, F_GLOB = 64, 32, 16
N_CORES = 8
GPC = N_GRAPHS // N_CORES  # graphs per core = 2

BF16 = mybir.dt.bfloat16
F32 = mybir.dt.float32
npbf16 = ml_dtypes.bfloat16

# ---------------------------------------------------------------------------
# Workaround: CoreV3 codegen rejects the TileContext final drain when it
# carries more than one semaphore wait. Split the waits across extra no-ops.
_MAX_WAITS = 1


_ENGINE_WAIT_LIMIT = 1
_SPLIT_ENGINES = None  # set lazily


def _split_excess_waits(nc):
    """CoreV3 codegen caps per-instruction sem waits. Move excess waits
    onto same-engine no-ops inserted immediately before the offender."""
    global _SPLIT_ENGINES
    if _SPLIT_ENGINES is None:
        ET = mybir.EngineType
        _SPLIT_ENGINES = {ET.PE, ET.Activation, ET.DVE, ET.SP, ET.Pool}
    ctr = [0]
    for bass_bb in nc.bb_map.values():
        bb = bass_bb.bb
        il = bb.instructions
        out = []
        changed = False
        for inst in il:
            si = inst.sync_info
            waits = list(si.on_wait) if (si and si.on_wait) else []
            if len(waits) > _ENGINE_WAIT_LIMIT and inst.engine in _SPLIT_ENGINES:
                head, keep = waits[:-_ENGINE_WAIT_LIMIT], waits[-_ENGINE_WAIT_LIMIT:]
                for i in range(0, len(head), _ENGINE_WAIT_LIMIT):
                    nop = mybir.InstNoOp(name=f"waitsplit-{ctr[0]}", ins=[], outs=[])
                    ctr[0] += 1
                    nop.engine = inst.engine
                    nop.sync_info = mybir.SyncInfo(
                        on_wait=head[i : i + _ENGINE_WAIT_LIMIT], on_update=[]
                    )
                    nc.register_instruction(nop, overwrite=True)
                    out.append(nop)
                inst.sync_info = mybir.SyncInfo(
                    on_wait=keep, on_update=list(si.on_update or [])
                )
                changed = True
            out.append(inst)
        if changed:
            bb.instructions = out


def _split_drain_and_barrier(self, tick_clock, wait_clock):
    nc = self.nc
    _split_excess_waits(nc)
    drain_inst = nc.sync.drain()
    wait_clock.add_sem_waits(
        drain_inst.ins, ScopedClock({None: tick_clock.global_clock})
    )
    mi = drain_inst.ins
    waits = list(mi.sync_info.on_wait) if (mi.sync_info and mi.sync_info.on_wait) else []
    if len(waits) > _MAX_WAITS:
        upd = list(mi.sync_info.on_update) if mi.sync_info.on_update else []
        mi.sync_info = mybir.SyncInfo(on_wait=waits[:_MAX_WAITS], on_update=upd)
        for i in range(_MAX_WAITS, len(waits), _MAX_WAITS):
            nop = nc.sync.nop(nofuse=True)
            nop.ins.sync_info = mybir.SyncInfo(
                on_wait=waits[i : i + _MAX_WAITS], on_update=[]
            )
    nc.all_engine_barrier()
    assert self.sems is not None
    popped = nc._tile_sem_poison_stack.pop()
    assert popped is self._sem_poison
    nc.clear_and_free_semaphores(list(self.sems.allocated().values()))
    nc.all_engine_barrier()


tile_mod.TileContext._drain_and_barrier = _split_drain_and_barrier


# ---------------------------------------------------------------------------
# Host-side graph partitioning / layout


def _pack_core(node_ids, degs, nt, cap_e):
    """LPT: place nodes (descending degree) onto the least-edge-loaded tile
    that still has node capacity. Returns per-tile node-id arrays, or None
    if some tile exceeds cap_e edges."""
    order = np.argsort(-degs, kind="stable")
    tiles_n = [[] for _ in range(nt)]
    tile_ncnt = np.zeros(nt, np.int64)
    tile_ecnt = np.zeros(nt, np.int64)
    for j in order:
        cand = np.where(tile_ncnt < 128)[0]
        if len(cand) == 0:
            return None
        t = cand[np.argmin(tile_ecnt[cand])]
        tiles_n[t].append(node_ids[j])
        tile_ncnt[t] += 1
        tile_ecnt[t] += degs[j]
    if (tile_ecnt > cap_e).any():
        return None
    return [np.array(t, dtype=np.int64) for t in tiles_n]


def _prepare(inputs):
    nf = np.asarray(inputs["node_feats"], np.float32)
    ef = np.asarray(inputs["edge_feats"], np.float32)
    glob = np.asarray(inputs["globals_"], np.float32)
    recv = np.asarray(inputs["receivers"]).astype(np.int64)
    ngraph = np.asarray(inputs["node_graph"]).astype(np.int64)

    cnt = np.bincount(recv, minlength=N_NODES).astype(np.int64)
    egraph = ngraph[recv]
    ncnt_g = np.bincount(ngraph, minlength=N_GRAPHS)
    ecnt_g = np.bincount(egraph, minlength=N_GRAPHS)

    node_core = ngraph // GPC
    edge_core = egraph // GPC

    core_nodes = [np.where(node_core == c)[0] for c in range(N_CORES)]
    NT = int(max((len(cn) + 127) // 128 for cn in core_nodes))

    packs = None
    K0 = max(1, int(max(np.bincount(edge_core, minlength=N_CORES)) + NT * 128 - 1)
             // (NT * 128))
    for k0 in range(K0, K0 + 12):
        trial = []
        ok = True
        for c in range(N_CORES):
            p = _pack_core(core_nodes[c], cnt[core_nodes[c]], NT, k0 * 128)
            if p is None:
                ok = False
                break
            trial.append(p)
        if ok:
            packs, K0 = trial, k0
            break
    assert packs is not None, "bin packing failed"

    NPAD = NT * 128
    EPAD = NT * K0 * 128

    # slot assignment per core
    w_np = {}
    slot_of_node = np.full(N_NODES, -1, np.int64)
    tile_of_node = np.full(N_NODES, -1, np.int64)
    in_maps = []
    for c in range(N_CORES):
        for t in range(NT):
            ids = packs[c][t]
            slot_of_node[ids] = t * 128 + np.arange(len(ids))
            tile_of_node[ids] = t

        # ---- edges
        eidx = np.where(edge_core == c)[0]
        et = tile_of_node[recv[eidx]]
        order = np.argsort(et, kind="stable")
        eidx = eidx[order]
        et = et[order]
        counts = np.bincount(et, minlength=NT)
        starts = np.concatenate([[0], np.cumsum(counts)[:-1]])
        off_in = np.arange(len(eidx)) - np.repeat(starts, counts)
        dst = et * (K0 * 128) + off_in
        assert (counts <= K0 * 128).all()

        eftT = np.zeros((33, EPAD), np.float32)
        eftT[:32, dst] = ef[eidx].T
        eftT[32, dst] = 1.0

        eg_loc = egraph[eidx] - c * GPC
        ghot = np.zeros((3, EPAD), np.float32)
        ghot[0, dst] = (eg_loc == 0)
        ghot[1, dst] = (eg_loc == 1)
        ghot[2, dst] = 1.0

        selidx = np.full(EPAD, -1.0, np.float32)
        selidx[dst] = (slot_of_node[recv[eidx]] % 128).astype(np.float32)
        # [NT, 128, K0] : chunk k, lane i  <- position (t*K0 + k)*128 + i
        sel3 = selidx.reshape(NT, K0, 128).transpose(0, 2, 1).copy()

        # ---- nodes
        slot_node = np.full(NPAD, -1, np.int64)
        for t in range(NT):
            ids = packs[c][t]
            slot_node[t * 128 : t * 128 + len(ids)] = ids
        valid = slot_node >= 0
        sn = np.where(valid, slot_node, 0)

        nftT = np.zeros((65, NPAD), np.float32)
        nftT[:64, valid] = nf[sn[valid]].T
        nftT[64, valid] = 1.0

        ng_loc = ngraph[sn] - c * GPC
        nhot = np.zeros((3, NPAD), np.float32)
        nhot[0] = valid * (ng_loc == 0)
        nhot[1] = valid * (ng_loc == 1)
        nhot[2] = valid * 1.0

        invc = np.zeros((NPAD, 1), np.float32)
        invc[valid, 0] = 1.0 / np.maximum(cnt[sn[valid]], 1)

        poolw = np.zeros((NPAD, 4), np.float32)
        for g in range(GPC):
            gid = c * GPC + g
            m = valid & (ng_loc == g)
            poolw[m, g] = 1.0 / max(ncnt_g[gid], 1)
            poolw[m, 2 + g] = cnt[sn[m]] / max(ecnt_g[gid], 1)

        globT = glob[c * GPC : (c + 1) * GPC].T.copy()  # [16, 2]

        in_maps.append(
            {
                "eft": eftT.astype(npbf16),
                "ghot": ghot.astype(npbf16),
                "selidx": sel3,
                "nft": nftT.astype(npbf16),
                "nhot": nhot.astype(npbf16),
                "invc": invc,
                "poolw": poolw.astype(npbf16),
                "globT": globT,
            }
        )

    # ---- replicated weights
    def bf(x):
        return np.ascontiguousarray(x).astype(npbf16)

    We1T = np.zeros((33, 256), np.float32)
    We1T[:32] = np.asarray(inputs["We1"], np.float32).T
    We1T[32] = np.asarray(inputs["be1"], np.float32)
    w_np["We1T"] = bf(We1T)

    We2 = np.asarray(inputs["We2"], np.float32)  # [128, 256]
    We2T = We2.T  # [256, 128]
    w_np["We2T"] = bf(np.concatenate([We2T[:128], We2T[128:]], axis=1))  # [128, 256]

    Wn1T = np.zeros((65, 256), np.float32)
    Wn1T[:64] = np.asarray(inputs["Wn1"], np.float32).T
    Wn1T[64] = np.asarray(inputs["bn1"], np.float32)
    w_np["Wn1T"] = bf(Wn1T)

    Win1T = np.asarray(inputs["Win1"], np.float32).T  # [256 fi, 256 fo]
    w_np["Win1T"] = bf(
        np.concatenate(
            [Win1T[:128, :128], Win1T[:128, 128:], Win1T[128:, :128], Win1T[128:, 128:]],
            axis=1,
        )
    )  # [128, 512] : cols b*256 + s*128

    Wn2T = np.asarray(inputs["Wn2"], np.float32).T  # [256, 128]
    w_np["Wn2T"] = bf(np.concatenate([Wn2T[:128], Wn2T[128:]], axis=1))  # [128, 256]
    w_np["Win2T"] = bf(np.asarray(inputs["Win2"], np.float32).T)  # [128, 128]

    w_np["Wg2T"] = np.asarray(inputs["Wg2"], np.float32).T.copy()  # [16, 128] f32
    w_np["Wng2T"] = np.asarray(inputs["Wng2"], np.float32).T.copy()
    w_np["be2r"] = bf(np.asarray(inputs["be2"], np.float32)[None, :])
    w_np["bn2r"] = bf(np.asarray(inputs["bn2"], np.float32)[None, :])

    w_np["WgnT"] = np.asarray(inputs["Wgn"], np.float32).T.copy()  # [128,128] f32
    w_np["WgeT"] = np.asarray(inputs["Wge"], np.float32).T.copy()
    w_np["WggT"] = np.asarray(inputs["Wgg"], np.float32).T.copy()  # [16, 128]
    w_np["bgr"] = np.asarray(inputs["bg"], np.float32)[None, :].copy()
    w_np["ones2"] = np.ones((1, 2), np.float32)
    w_np["iota"] = np.broadcast_to(
        np.arange(128, dtype=np.float32), (128, 128)
    ).copy()
    w_np["ident"] = np.eye(128, dtype=npbf16)
    w_np["ident2"] = np.eye(2, dtype=np.float32)

    for m in in_maps:
        m.update(w_np)
    return in_maps, NT, K0


# ---------------------------------------------------------------------------
# Device program (identical on all cores)


def _build(NT, K0):
    nc = bass.Bass()
    NPAD = NT * 128
    EPAD = NT * K0 * 128
    CW = K0 * 128  # edge columns per node-tile

    d_eft = nc.dram_tensor("eft", [33, EPAD], BF16, kind="ExternalInput")
    d_ghot = nc.dram_tensor("ghot", [3, EPAD], BF16, kind="ExternalInput")
    d_sel = nc.dram_tensor("selidx", [NT, 128, K0], F32, kind="ExternalInput")
    d_nft = nc.dram_tensor("nft", [65, NPAD], BF16, kind="ExternalInput")
    d_nhot = nc.dram_tensor("nhot", [3, NPAD], BF16, kind="ExternalInput")
    d_invc = nc.dram_tensor("invc", [NPAD, 1], F32, kind="ExternalInput")
    d_poolw = nc.dram_tensor("poolw", [NPAD, 4], BF16, kind="ExternalInput")
    d_globT = nc.dram_tensor("globT", [16, 2], F32, kind="ExternalInput")

    d_We1T = nc.dram_tensor("We1T", [33, 256], BF16, kind="ExternalInput")
    d_We2T = nc.dram_tensor("We2T", [128, 256], BF16, kind="ExternalInput")
    d_Wn1T = nc.dram_tensor("Wn1T", [65, 256], BF16, kind="ExternalInput")
    d_Win1T = nc.dram_tensor("Win1T", [128, 512], BF16, kind="ExternalInput")
    d_Wn2T = nc.dram_tensor("Wn2T", [128, 256], BF16, kind="ExternalInput")
    d_Win2T = nc.dram_tensor("Win2T", [128, 128], BF16, kind="ExternalInput")
    d_Wg2T = nc.dram_tensor("Wg2T", [16, 128], F32, kind="ExternalInput")
    d_Wng2T = nc.dram_tensor("Wng2T", [16, 128], F32, kind="ExternalInput")
    d_be2r = nc.dram_tensor("be2r", [1, 128], BF16, kind="ExternalInput")
    d_bn2r = nc.dram_tensor("bn2r", [1, 128], BF16, kind="ExternalInput")
    d_WgnT = nc.dram_tensor("WgnT", [128, 128], F32, kind="ExternalInput")
    d_WgeT = nc.dram_tensor("WgeT", [128, 128], F32, kind="ExternalInput")
    d_WggT = nc.dram_tensor("WggT", [16, 128], F32, kind="ExternalInput")
    d_bgr = nc.dram_tensor("bgr", [1, 128], F32, kind="ExternalInput")
    d_ones2 = nc.dram_tensor("ones2", [1, 2], F32, kind="ExternalInput")
    d_iota = nc.dram_tensor("iota", [128, 128], F32, kind="ExternalInput")
    d_ident = nc.dram_tensor("ident", [128, 128], BF16, kind="ExternalInput")
    d_ident2 = nc.dram_tensor("ident2", [2, 2], F32, kind="ExternalInput")

    d_out = nc.dram_tensor("out", [128, 2], F32, kind="ExternalOutput")

    Relu = mybir.ActivationFunctionType.Relu
    Copy = mybir.ActivationFunctionType.Copy

    with tile.TileContext(nc) as tc:
        with tc.tile_pool(name="wp", bufs=1) as wp:
            def wtile(dram, shape, dt):
                t = wp.tile(shape, dt, tag=dram.name)
                nc.sync.dma_start(t[:], dram[:])
                return t

            We1T = wtile(d_We1T, [33, 256], BF16)
            We2T = wtile(d_We2T, [128, 256], BF16)
            Wn1T = wtile(d_Wn1T, [65, 256], BF16)
            Win1T = wtile(d_Win1T, [128, 512], BF16)
            Wn2T = wtile(d_Wn2T, [128, 256], BF16)
            Win2T = wtile(d_Win2T, [128, 128], BF16)
            Wg2T = wtile(d_Wg2T, [16, 128], F32)
            Wng2T = wtile(d_Wng2T, [16, 128], F32)
            WgnT = wtile(d_WgnT, [128, 128], F32)
            WgeT = wtile(d_WgeT, [128, 128], F32)
            WggT = wtile(d_WggT, [16, 128], F32)
            bgr = wtile(d_bgr, [1, 128], F32)
            ones2 = wtile(d_ones2, [1, 2], F32)
            iota = wtile(d_iota, [128, 128], F32)
            ident = wtile(d_ident, [128, 128], BF16)
            ident2 = wtile(d_ident2, [2, 2], F32)
            globT = wtile(d_globT, [16, 2], F32)

            aggall = wp.tile([128, 384 * NT], BF16, tag="aggall")
            g2aug = wp.tile([3, 128], BF16, tag="g2aug")
            gnaug = wp.tile([3, 128], BF16, tag="gnaug")

            # --- per-core global projections gb = globals @ Wg2.T etc.
            with tc.tile_pool(name="psg", bufs=1, space=bass.MemorySpace.PSUM) as psg:
                pg = psg.tile([2, 256], F32, tag="pg")
                nc.tensor.matmul(pg[:, 0:128], globT[:], Wg2T[:], start=True, stop=True)
                nc.tensor.matmul(pg[:, 128:256], globT[:], Wng2T[:], start=True, stop=True)
                nc.scalar.activation(g2aug[0:2, :], pg[:, 0:128], Copy)
                nc.scalar.activation(gnaug[0:2, :], pg[:, 128:256], Copy)
                nc.sync.dma_start(g2aug[2:3, :], d_be2r[:])
                nc.sync.dma_start(gnaug[2:3, :], d_bn2r[:])

            # ----------------- edge phase -----------------
            with tc.tile_pool(name="ep", bufs=3) as ep, \
                 tc.tile_pool(name="esb", bufs=6) as esb, \
                 tc.tile_pool(name="psA", bufs=2, space=bass.MemorySpace.PSUM) as psA, \
                 tc.tile_pool(name="psB", bufs=2, space=bass.MemorySpace.PSUM) as psB, \
                 tc.tile_pool(name="psC", bufs=2, space=bass.MemorySpace.PSUM) as psC, \
                 tc.tile_pool(name="psAgg", bufs=2, space=bass.MemorySpace.PSUM) as psAgg:
                for t in range(NT):
                    eftt = ep.tile([33, CW], BF16, tag="eftt")
                    nc.sync.dma_start(eftt[:], d_eft[:, t * CW : (t + 1) * CW])
                    ght = ep.tile([3, CW], BF16, tag="ght")
                    nc.sync.dma_start(ght[:], d_ghot[:, t * CW : (t + 1) * CW])
                    sidx = ep.tile([128, K0], F32, tag="sidx")
                    nc.sync.dma_start(sidx[:], d_sel[t])
                    invc_t = ep.tile([128, 1], F32, tag="invc")
                    nc.sync.dma_start(invc_t[:], d_invc[t * 128 : (t + 1) * 128, :])

                    # one-hot selector columns for the whole tile, up front
                    ohall = esb.tile([128, CW], BF16, tag="ohall", bufs=2)
                    for k in range(K0):
                        nc.vector.tensor_scalar(
                            ohall[:, k * 128 : (k + 1) * 128], iota[:],
                            sidx[:, k : k + 1], None,
                            op0=mybir.AluOpType.is_equal,
                        )

                    pagg = psAgg.tile([128, 384], F32, tag="pagg")
                    pairs = [(p, min(p + 2, K0)) for p in range(0, K0, 2)]

                    def stage_a(p0, p1):
                        """e1T + e1 for chunks [p0, p1): produce e1T (bf16) and
                        the e1 halves of the ef tiles."""
                        g2 = slice(p0 * 128, p1 * 128)
                        gw = g2.stop - g2.start
                        pe1T = psB.tile([128, 512], F32, tag="pe1T")
                        nc.tensor.matmul(pe1T[:, 0:gw], We1T[:, 0:128],
                                         eftt[:, g2], start=True, stop=True)
                        nc.tensor.matmul(pe1T[:, 256 : 256 + gw], We1T[:, 128:256],
                                         eftt[:, g2], start=True, stop=True)
                        e1T = esb.tile([128, 512], BF16, tag="e1T")
                        nc.vector.tensor_scalar_max(e1T[:], pe1T[:], 0.0)
                        efs = []
                        for k in range(p0, p1):
                            sl = slice(k * 128, (k + 1) * 128)
                            pe1 = psA.tile([128, 256], F32, tag="pe1")
                            nc.tensor.matmul(pe1[:], eftt[:, sl], We1T[:],
                                             start=True, stop=True)
                            ef = esb.tile([128, 384], BF16, tag="ef")
                            nc.scalar.activation(ef[:, 0:256], pe1[:], Relu)
                            efs.append(ef)
                        return e1T, efs

                    def stage_b(p0, p1, e1T, efs):
                        """e2 + aggregation for chunks [p0, p1), consuming the
                        e1T produced a pair earlier."""
                        for k in range(p0, p1):
                            sl = slice(k * 128, (k + 1) * 128)
                            ko = (k - p0) * 128
                            ef = efs[k - p0]
                            pe2 = psC.tile([128, 128], F32, tag="pe2")
                            nc.tensor.matmul(pe2[:], e1T[:, ko : ko + 128],
                                             We2T[:, 0:128], start=True, stop=False)
                            nc.tensor.matmul(pe2[:], e1T[:, 256 + ko : 256 + ko + 128],
                                             We2T[:, 128:256], start=False, stop=False)
                            nc.tensor.matmul(pe2[:], ght[:, sl], g2aug[:],
                                             start=False, stop=True)
                            nc.vector.tensor_scalar_max(ef[:, 256:384], pe2[:], 0.0)
                            nc.tensor.matmul(pagg[:], ohall[:, sl], ef[:],
                                             start=(k == 0), stop=(k == K0 - 1))

                    prev = None
                    for (p0, p1) in pairs:
                        cur = (p0, p1, *stage_a(p0, p1))
                        if prev is not None:
                            stage_b(*prev)
                        prev = cur
                    stage_b(*prev)

                    nc.scalar.activation(
                        aggall[:, t * 384 : (t + 1) * 384], pagg[:], Copy,
                        scale=invc_t[:],
                    )

            # ----------------- node phase -----------------
            with tc.tile_pool(name="np_", bufs=2) as np_, \
                 tc.tile_pool(name="nsb", bufs=3) as nsb, \
                 tc.tile_pool(name="npsA", bufs=2, space=bass.MemorySpace.PSUM) as npsA, \
                 tc.tile_pool(name="npsB", bufs=2, space=bass.MemorySpace.PSUM) as npsB, \
                 tc.tile_pool(name="npsC", bufs=2, space=bass.MemorySpace.PSUM) as npsC, \
                 tc.tile_pool(name="npsP", bufs=1, space=bass.MemorySpace.PSUM) as npsP:
                ppN = npsP.tile([2, 128], F32, tag="ppN")
                ppE = npsP.tile([2, 128], F32, tag="ppE")
                for t in range(NT):
                    aggsl = aggall[:, t * 384 : (t + 1) * 384]
                    pT = npsA.tile([128, 384], BF16, tag="pT")
                    nc.tensor.transpose(pT[:, 0:128], aggsl[:, 0:128], ident[:])
                    nc.tensor.transpose(pT[:, 128:256], aggsl[:, 128:256], ident[:])
                    nc.tensor.transpose(pT[:, 256:384], aggsl[:, 256:384], ident[:])
                    aggT = nsb.tile([128, 384], BF16, tag="aggT")
                    nc.vector.tensor_copy(aggT[:], pT[:])

                    nftt = np_.tile([65, 128], BF16, tag="nftt")
                    nc.sync.dma_start(nftt[:], d_nft[:, t * 128 : (t + 1) * 128])
                    nht = np_.tile([3, 128], BF16, tag="nht")
                    nc.sync.dma_start(nht[:], d_nhot[:, t * 128 : (t + 1) * 128])
                    pw = np_.tile([128, 4], BF16, tag="pw")
                    nc.sync.dma_start(pw[:], d_poolw[t * 128 : (t + 1) * 128, :])

                    pn1 = npsB.tile([128, 256], F32, tag="pn1")
                    for s in (0, 1):
                        ssl = slice(s * 128, (s + 1) * 128)
                        nc.tensor.matmul(pn1[:, ssl], Wn1T[:, ssl], nftt[:], start=True, stop=False)
                        nc.tensor.matmul(pn1[:, ssl], Win1T[:, s * 128 : s * 128 + 128],
                                         aggT[:, 0:128], start=False, stop=False)
                        nc.tensor.matmul(pn1[:, ssl], Win1T[:, 256 + s * 128 : 256 + s * 128 + 128],
                                         aggT[:, 128:256], start=False, stop=True)
                    n1T = nsb.tile([128, 256], BF16, tag="n1T")
                    nc.scalar.activation(n1T[:], pn1[:], Relu)

                    pn2 = npsC.tile([128, 128], F32, tag="pn2")
                    nc.tensor.matmul(pn2[:], n1T[:, 0:128], Wn2T[:, 0:128], start=True, stop=False)
                    nc.tensor.matmul(pn2[:], n1T[:, 128:256], Wn2T[:, 128:256], start=False, stop=False)
                    nc.tensor.matmul(pn2[:], aggT[:, 256:384], Win2T[:], start=False, stop=False)
                    nc.tensor.matmul(pn2[:], nht[:], gnaug[:], start=False, stop=True)
                    n2 = nsb.tile([128, 128], BF16, tag="n2")
                    nc.scalar.activation(n2[:], pn2[:], Relu)

                    nc.tensor.matmul(ppN[:], pw[:, 0:2], n2[:],
                                     start=(t == 0), stop=(t == NT - 1))
                    nc.tensor.matmul(ppE[:], pw[:, 2:4], aggsl[:, 256:384],
                                     start=(t == 0), stop=(t == NT - 1))

                # ----------------- final projection -----------------
                navg = nsb.tile([2, 128], F32, tag="navg")
                nc.scalar.activation(navg[:], ppN[:], Copy)
                eavg = nsb.tile([2, 128], F32, tag="eavg")
                nc.scalar.activation(eavg[:], ppE[:], Copy)

                ptr2 = npsA.tile([128, 4], F32, tag="pT")
                nc.tensor.transpose(ptr2[:, 0:2], navg[:], ident2[:])
                nc.tensor.transpose(ptr2[:, 2:4], eavg[:], ident2[:])
                nt2 = nsb.tile([128, 4], F32, tag="nt2")
                nc.scalar.activation(nt2[:], ptr2[:], Copy)

                pout = npsC.tile([128, 2], F32, tag="pn2")
                nc.tensor.matmul(pout[:], WgnT[:], nt2[:, 0:2], start=True, stop=False)
                nc.tensor.matmul(pout[:], WgeT[:], nt2[:, 2:4], start=False, stop=False)
                nc.tensor.matmul(pout[:], WggT[:], globT[:], start=False, stop=False)
                nc.tensor.matmul(pout[:], bgr[:], ones2[:], start=False, stop=True)
                outsb = nsb.tile([128, 2], F32, tag="outsb")
                nc.scalar.activation(outsb[:], pout[:], Copy)
                nc.sync.dma_start(d_out[:], outsb[:])

    return nc


_CACHE = {}


def _get_nc(NT, K0):
    key = (NT, K0)
    if key not in _CACHE:
        _CACHE[key] = _build(NT, K0)
    return _CACHE[key]


def _run(inputs, trace=False):
    in_maps, NT, K0 = _prepare(inputs)
    nc = _get_nc(NT, K0)
    res = run_bass_kernel_spmd(nc, in_maps, list(range(N_CORES)), trace=trace)
    out = np.zeros((N_GRAPHS, 128), np.float32)
    for c in range(N_CORES):
        r = np.asarray(res.results[c]["out"], np.float32)
        out[GPC * c] = r[:, 0]
        out[GPC * c + 1] = r[:, 1]
    return out, res


def kernel(**inputs):
    out, _ = _run(inputs, trace=False)
    return out


def kernel_traced(**inputs):
    return _run(inputs, trace=True)



# revision 19
# speedup vs baseline: 1.1048x; 1.1048x over previous
"""Trainium2 Bass kernel for a 2-layer GraphNetwork (gnn_message_passing).

Strategy (v2):
  - 16 graphs partitioned across 8 cores (2 graphs/core); every edge's
    receiver lives on its own core, so all segment reductions are
    core-local. [16,128] output rows are gathered on the host.
  - Per core, nodes are bin-packed into NT tiles of 128 slots; each
    tile's incoming edges are padded to K0 chunks of 128, processed in
    PAIRS (even chunk on PE row-band 0, odd chunk on band 64) so the
    K=33 edge-layer matmuls run 2x via tensor-engine row tiling.
  - The e2 projection (K=256) and the one-hot segment-sum aggregation
    (contraction over 256 edges/pair) run as fp8e4 DoubleRow matmuls.
    One-hot selector blocks are prebuilt on the host and DMA'd in fp8.
  - Biases/global projections are folded into extra stationary rows
    (ones / graph-one-hot) so no separate bias adds are needed.
  - fp32 PSUM accumulation everywhere; final stage fp32.
"""

import numpy as np
import ml_dtypes

import concourse.bass as bass
import concourse.tile as tile_mod
from concourse import tile
from concourse.bass_utils import run_bass_kernel_spmd
from concourse.vector_clock import ScopedClock

mybir = bass.mybir

N_NODES, N_EDGES, N_GRAPHS = 20000, 320000, 16
F_NODE, F_EDGE, F_GLOB = 64, 32, 16
N_CORES = 8
GPC = N_GRAPHS // N_CORES  # graphs per core = 2

BF16 = mybir.dt.bfloat16
F32 = mybir.dt.float32
FP8 = mybir.dt.float8e4
npbf16 = ml_dtypes.bfloat16
npfp8 = mybir.dt.np(FP8)
DR = mybir.MatmulPerfMode.DoubleRow
Relu = None  # set after import
Copy = None

# ---------------------------------------------------------------------------
# Workaround: CoreV3 codegen rejects the TileContext final drain when it
# carries more than one semaphore wait. Split the waits across extra no-ops.
_MAX_WAITS = 1
_ENGINE_WAIT_LIMIT = 1
_SPLIT_ENGINES = None  # set lazily


def _split_excess_waits(nc):
    """CoreV3 codegen caps per-instruction sem waits. Move excess waits
    onto same-engine no-ops inserted immediately before the offender."""
    global _SPLIT_ENGINES
    if _SPLIT_ENGINES is None:
        ET = mybir.EngineType
        _SPLIT_ENGINES = {ET.PE, ET.Activation, ET.DVE, ET.SP, ET.Pool}
    ctr = [0]
    for bass_bb in nc.bb_map.values():
        bb = bass_bb.bb
        il = bb.instructions
        out = []
        changed = False
        for inst in il:
            si = inst.sync_info
            waits = list(si.on_wait) if (si and si.on_wait) else []
            if len(waits) > _ENGINE_WAIT_LIMIT and inst.engine in _SPLIT_ENGINES:
                head, keep = waits[:-_ENGINE_WAIT_LIMIT], waits[-_ENGINE_WAIT_LIMIT:]
                for i in range(0, len(head), _ENGINE_WAIT_LIMIT):
                    nop = mybir.InstNoOp(name=f"waitsplit-{ctr[0]}", ins=[], outs=[])
                    ctr[0] += 1
                    nop.engine = inst.engine
                    nop.sync_info = mybir.SyncInfo(
                        on_wait=head[i : i + _ENGINE_WAIT_LIMIT], on_update=[]
                    )
                    nc.register_instruction(nop, overwrite=True)
                    out.append(nop)
                inst.sync_info = mybir.SyncInfo(
                    on_wait=keep, on_update=list(si.on_update or [])
                )
                changed = True
            out.append(inst)
        if changed:
            bb.instructions = out


def _split_drain_and_barrier(self, tick_clock, wait_clock):
    nc = self.nc
    _split_excess_waits(nc)
    drain_inst = nc.sync.drain()
    wait_clock.add_sem_waits(
        drain_inst.ins, ScopedClock({None: tick_clock.global_clock})
    )
    mi = drain_inst.ins
    waits = list(mi.sync_info.on_wait) if (mi.sync_info and mi.sync_info.on_wait) else []
    if len(waits) > _MAX_WAITS:
        upd = list(mi.sync_info.on_update) if mi.sync_info.on_update else []
        mi.sync_info = mybir.SyncInfo(on_wait=waits[:_MAX_WAITS], on_update=upd)
        for i in range(_MAX_WAITS, len(waits), _MAX_WAITS):
            nop = nc.sync.nop(nofuse=True)
            nop.ins.sync_info = mybir.SyncInfo(
                on_wait=waits[i : i + _MAX_WAITS], on_update=[]
            )
    nc.all_engine_barrier()
    assert self.sems is not None
    popped = nc._tile_sem_poison_stack.pop()
    assert popped is self._sem_poison
    nc.clear_and_free_semaphores(list(self.sems.allocated().values()))
    nc.all_engine_barrier()


tile_mod.TileContext._drain_and_barrier = _split_drain_and_barrier


# ---------------------------------------------------------------------------
# Host-side graph partitioning / layout


def _pack_core(node_ids, degs, nt, cap_e):
    """LPT: place nodes (descending degree) onto the least-edge-loaded tile
    that still has node capacity. Returns per-tile node-id arrays, or None
    if some tile exceeds cap_e edges."""
    order = np.argsort(-degs, kind="stable")
    tiles_n = [[] for _ in range(nt)]
    tile_ncnt = np.zeros(nt, np.int64)
    tile_ecnt = np.zeros(nt, np.int64)
    for j in order:
        cand = np.where(tile_ncnt < 128)[0]
        if len(cand) == 0:
            return None
        t = cand[np.argmin(tile_ecnt[cand])]
        tiles_n[t].append(node_ids[j])
        tile_ncnt[t] += 1
        tile_ecnt[t] += degs[j]
    if (tile_ecnt > cap_e).any():
        return None
    return [np.array(t, dtype=np.int64) for t in tiles_n]


def _prepare(inputs):
    nf = np.asarray(inputs["node_feats"], np.float32)
    ef = np.asarray(inputs["edge_feats"], np.float32)
    glob = np.asarray(inputs["globals_"], np.float32)
    recv = np.asarray(inputs["receivers"]).astype(np.int64)
    ngraph = np.asarray(inputs["node_graph"]).astype(np.int64)

    cnt = np.bincount(recv, minlength=N_NODES).astype(np.int64)
    egraph = ngraph[recv]
    ncnt_g = np.bincount(ngraph, minlength=N_GRAPHS)
    ecnt_g = np.bincount(egraph, minlength=N_GRAPHS)

    node_core = ngraph // GPC
    edge_core = egraph // GPC

    core_nodes = [np.where(node_core == c)[0] for c in range(N_CORES)]
    NT = int(max((len(cn) + 127) // 128 for cn in core_nodes))

    packs = None
    K0 = max(1, int(max(np.bincount(edge_core, minlength=N_CORES)) + NT * 128 - 1)
             // (NT * 128))
    if K0 % 2:
        K0 += 1
    for k0 in range(K0, K0 + 13, 2):
        trial = []
        ok = True
        for c in range(N_CORES):
            p = _pack_core(core_nodes[c], cnt[core_nodes[c]], NT, k0 * 128)
            if p is None:
                ok = False
                break
            trial.append(p)
        if ok:
            packs, K0 = trial, k0
            break
    assert packs is not None, "bin packing failed"

    NPAD = NT * 128
    EPAD = NT * K0 * 128
    NPAIR = NT * K0 // 2

    # --- replicated weights / folded projections (per-core globals differ)
    We1T = np.zeros((33, 256), np.float32)
    We1T[:32] = np.asarray(inputs["We1"], np.float32).T
    We1T[32] = np.asarray(inputs["be1"], np.float32)
    We1T2 = np.zeros((128, 256), np.float32)
    We1T2[0:33] = We1T
    We1T2[64:97] = We1T

    We2T = np.asarray(inputs["We2"], np.float32).T  # [256, 128]
    We2DR = np.concatenate([We2T[:128], We2T[128:]], axis=1)  # [128, 256]

    Wn1T = np.asarray(inputs["Wn1"], np.float32).T  # [64, 256]
    bn1c = np.asarray(inputs["bn1"], np.float32).reshape(2, 128).T.copy()  # [128,2]

    Win1T = np.asarray(inputs["Win1"], np.float32).T  # [256 fi, 256 fo]
    # DR layout per fo-slice s: [128, 2, 128] -> cols 256*s + 128*i + fo
    Win1DR = np.zeros((128, 512), np.float32)
    for s in range(2):
        for i in range(2):
            Win1DR[:, 256 * s + 128 * i : 256 * s + 128 * i + 128] = \
                Win1T[128 * i : 128 * i + 128, 128 * s : 128 * s + 128]

    Wn2T = np.asarray(inputs["Wn2"], np.float32).T  # [256, 128]
    Wn2DR = np.concatenate([Wn2T[:128], Wn2T[128:]], axis=1)  # [128, 256]
    Win2T = np.asarray(inputs["Win2"], np.float32).T  # [128, 128]

    Wg2T = np.asarray(inputs["Wg2"], np.float32).T  # [16, 128]
    Wng2T = np.asarray(inputs["Wng2"], np.float32).T
    be2 = np.asarray(inputs["be2"], np.float32)
    bn2 = np.asarray(inputs["bn2"], np.float32)

    w_np = {
        "We1T2": We1T2.astype(npbf16),
        "We2DR": We2DR.astype(npfp8),
        "Wn1T": Wn1T.astype(npbf16),
        "bn1c": bn1c,
        "Win1DR": Win1DR.astype(npfp8),
        "Wn2DR": Wn2DR.astype(npfp8),
        "Win2f8": Win2T.astype(npfp8),
        "WgnT": np.asarray(inputs["Wgn"], np.float32).T.copy(),
        "WgeT": np.asarray(inputs["Wge"], np.float32).T.copy(),
        "WggT": np.asarray(inputs["Wgg"], np.float32).T.copy(),
        "bgr": np.asarray(inputs["bg"], np.float32)[None, :].copy(),
        "ones2": np.ones((1, 2), np.float32),
        "ident": np.eye(128, dtype=npbf16),
        "ident2": np.eye(2, dtype=np.float32),
    }

    slot_of_node = np.full(N_NODES, -1, np.int64)
    tile_of_node = np.full(N_NODES, -1, np.int64)
    in_maps = []
    for c in range(N_CORES):
        for t in range(NT):
            ids = packs[c][t]
            slot_of_node[ids] = t * 128 + np.arange(len(ids))
            tile_of_node[ids] = t

        # ---- edges: assign slots (grouped by receiver tile)
        eidx = np.where(edge_core == c)[0]
        et = tile_of_node[recv[eidx]]
        order = np.argsort(et, kind="stable")
        eidx = eidx[order]
        et = et[order]
        counts = np.bincount(et, minlength=NT)
        starts = np.concatenate([[0], np.cumsum(counts)[:-1]])
        off_in = np.arange(len(eidx)) - np.repeat(starts, counts)
        dst = et * (K0 * 128) + off_in
        assert (counts <= K0 * 128).all()

        # edge-feature stationary block: rows 0:32 feat, 32 ones, 33 isg0, 34 isg1
        eg_loc = egraph[eidx] - c * GPC
        arr = np.zeros((35, EPAD), np.float32)
        arr[:32, dst] = ef[eidx].T
        arr[32, dst] = 1.0
        arr[33, dst] = (eg_loc == 0)
        arr[34, dst] = (eg_loc == 1)
        a4 = arr.reshape(35, NPAIR, 2, 128)
        eft2 = np.zeros((128, NPAIR * 128), np.float32)
        eft2.reshape(128, NPAIR, 128)[0:35] = a4[:, :, 0, :].transpose(0, 1, 2)
        eft2.reshape(128, NPAIR, 128)[64:99] = a4[:, :, 1, :].transpose(0, 1, 2)

        # one-hot selectors, fp8, DR pair layout: [128, pair*256 + 128*i + n]
        sel = np.full(EPAD, -1, np.int64)
        sel[dst] = slot_of_node[recv[eidx]] % 128
        oh = np.zeros((EPAD, 128), np.float32)
        vmask = sel >= 0
        oh[np.where(vmask)[0], sel[vmask]] = 1.0
        # oh[(pair,i,p), n] -> oh2[p, pair*256 + 128*i + n]
        oh2 = (
            oh.reshape(NPAIR, 2, 128, 128)
            .transpose(2, 0, 1, 3)
            .reshape(128, NPAIR * 256)
        )

        # per-core global projections folded w/ biases, dup at band rows
        g2aug = np.zeros((128, 128), np.float32)
        gl = glob[c * GPC : (c + 1) * GPC]  # [2, 16]
        gp = gl @ Wg2T  # [2, 128]
        # ghot stationary rows sit at partitions 32:35 / 96:99, so the
        # moving operand must live in the same 32-row quadrants
        g2aug[32] = be2
        g2aug[33:35] = gp
        g2aug[96] = be2
        g2aug[97:99] = gp
        gn = gl @ Wng2T.reshape(16, 128)
        gnaug = np.zeros((3, 128), np.float32)
        gnaug[0:2] = gn
        gnaug[2] = bn2

        # ---- nodes
        slot_node = np.full(NPAD, -1, np.int64)
        for t in range(NT):
            ids = packs[c][t]
            slot_node[t * 128 : t * 128 + len(ids)] = ids
        valid = slot_node >= 0
        sn = np.where(valid, slot_node, 0)

        nft = np.zeros((64, NPAD), np.float32)
        nft[:, valid] = nf[sn[valid]].T

        ng_loc = ngraph[sn] - c * GPC
        nhot = np.zeros((3, NPAD), np.float32)
        nhot[0] = valid * (ng_loc == 0)
        nhot[1] = valid * (ng_loc == 1)
        nhot[2] = valid * 1.0

        invc2 = np.zeros((NPAD, 1), np.float32)
        invc2[valid, 0] = 1.0 / np.maximum(cnt[sn[valid]], 1)
        invc2 = invc2.reshape(NT, 128).T.copy()  # [128, NT]

        poolw = np.zeros((NPAD, 4), np.float32)
        for g in range(GPC):
            gid = c * GPC + g
            m = valid & (ng_loc == g)
            poolw[m, g] = 1.0 / max(ncnt_g[gid], 1)
            poolw[m, 2 + g] = cnt[sn[m]] / max(ecnt_g[gid], 1)

        globT = glob[c * GPC : (c + 1) * GPC].T.copy()  # [16, 2]

        m = {
            "eft2": eft2.astype(npbf16),
            "oh2": oh2.astype(npfp8),
            "g2aug": g2aug.astype(npbf16),
            "gnaug": gnaug.astype(npbf16),
            "nft": nft.astype(npbf16),
            "nhot": nhot.astype(npbf16),
            "invc2": invc2,
            "poolw": poolw.astype(npbf16),
            "globT": globT,
        }
        m.update(w_np)
        in_maps.append(m)

    return in_maps, NT, K0


# ---------------------------------------------------------------------------
# Device program (identical on all cores)


def _build(NT, K0):
    global Relu, Copy
    Relu = mybir.ActivationFunctionType.Relu
    Copy = mybir.ActivationFunctionType.Copy

    nc = bass.Bass()
    NPAD = NT * 128
    EPAD = NT * K0 * 128
    NPAIR = NT * K0 // 2
    PPT = K0 // 2  # pairs per tile
    CW2 = PPT * 128  # eft2 cols per tile
    OW = PPT * 256  # oh2 cols per tile

    d_eft2 = nc.dram_tensor("eft2", [128, NPAIR * 128], BF16, kind="ExternalInput")
    d_oh2 = nc.dram_tensor("oh2", [128, NPAIR * 256], FP8, kind="ExternalInput")
    d_g2aug = nc.dram_tensor("g2aug", [128, 128], BF16, kind="ExternalInput")
    d_gnaug = nc.dram_tensor("gnaug", [3, 128], BF16, kind="ExternalInput")
    d_nft = nc.dram_tensor("nft", [64, NPAD], BF16, kind="ExternalInput")
    d_nhot = nc.dram_tensor("nhot", [3, NPAD], BF16, kind="ExternalInput")
    d_invc2 = nc.dram_tensor("invc2", [128, NT], F32, kind="ExternalInput")
    d_poolw = nc.dram_tensor("poolw", [NPAD, 4], BF16, kind="ExternalInput")
    d_globT = nc.dram_tensor("globT", [16, 2], F32, kind="ExternalInput")

    d_We1T2 = nc.dram_tensor("We1T2", [128, 256], BF16, kind="ExternalInput")
    d_We2DR = nc.dram_tensor("We2DR", [128, 256], FP8, kind="ExternalInput")
    d_Wn1T = nc.dram_tensor("Wn1T", [64, 256], BF16, kind="ExternalInput")
    d_bn1c = nc.dram_tensor("bn1c", [128, 2], F32, kind="ExternalInput")
    d_Win1DR = nc.dram_tensor("Win1DR", [128, 512], FP8, kind="ExternalInput")
    d_Wn2DR = nc.dram_tensor("Wn2DR", [128, 256], FP8, kind="ExternalInput")
    d_Win2f8 = nc.dram_tensor("Win2f8", [128, 128], FP8, kind="ExternalInput")
    d_WgnT = nc.dram_tensor("WgnT", [128, 128], F32, kind="ExternalInput")
    d_WgeT = nc.dram_tensor("WgeT", [128, 128], F32, kind="ExternalInput")
    d_WggT = nc.dram_tensor("WggT", [16, 128], F32, kind="ExternalInput")
    d_bgr = nc.dram_tensor("bgr", [1, 128], F32, kind="ExternalInput")
    d_ones2 = nc.dram_tensor("ones2", [1, 2], F32, kind="ExternalInput")
    d_ident = nc.dram_tensor("ident", [128, 128], BF16, kind="ExternalInput")
    d_ident2 = nc.dram_tensor("ident2", [2, 2], F32, kind="ExternalInput")

    d_out = nc.dram_tensor("out", [128, 2], F32, kind="ExternalOutput")

    def r3(ap, blk):
        # [128, 2*blk] -> [128, 2, blk] DoubleRow view
        return ap.rearrange("p (a b) -> p a b", a=2, b=blk)

    with tile.TileContext(nc) as tc:
        with tc.tile_pool(name="wp", bufs=1) as wp:
            def wtile(dram, shape, dt):
                t = wp.tile(shape, dt, tag=dram.name)
                nc.sync.dma_start(t[:], dram[:])
                return t

            We1T2 = wtile(d_We1T2, [128, 256], BF16)
            We2DR = wtile(d_We2DR, [128, 256], FP8)
            g2aug = wtile(d_g2aug, [128, 128], BF16)
            gnaug = wtile(d_gnaug, [3, 128], BF16)
            Wn1T = wtile(d_Wn1T, [64, 256], BF16)
            bn1c = wtile(d_bn1c, [128, 2], F32)
            Win1DR = wtile(d_Win1DR, [128, 512], FP8)
            Wn2DR = wtile(d_Wn2DR, [128, 256], FP8)
            Win2f8 = wtile(d_Win2f8, [128, 128], FP8)
            WgnT = wtile(d_WgnT, [128, 128], F32)
            WgeT = wtile(d_WgeT, [128, 128], F32)
            WggT = wtile(d_WggT, [16, 128], F32)
            bgr = wtile(d_bgr, [1, 128], F32)
            ones2 = wtile(d_ones2, [1, 2], F32)
            ident = wtile(d_ident, [128, 128], BF16)
            ident2 = wtile(d_ident2, [2, 2], F32)
            globT = wtile(d_globT, [16, 2], F32)
            invc2 = wtile(d_invc2, [128, NT], F32)
            nhot = wtile(d_nhot, [3, NPAD], BF16)

            with tc.tile_pool(name="ep", bufs=2) as ep, \
                 tc.tile_pool(name="esb", bufs=3) as esb, \
                 tc.tile_pool(name="nsb", bufs=2) as nsb, \
                 tc.tile_pool(name="psE", bufs=1, space=bass.MemorySpace.PSUM) as psE, \
                 tc.tile_pool(name="psO", bufs=1, space=bass.MemorySpace.PSUM) as psO, \
                 tc.tile_pool(name="ps2E", bufs=1, space=bass.MemorySpace.PSUM) as ps2E, \
                 tc.tile_pool(name="ps2O", bufs=1, space=bass.MemorySpace.PSUM) as ps2O, \
                 tc.tile_pool(name="psAgg", bufs=1, space=bass.MemorySpace.PSUM) as psAgg, \
                 tc.tile_pool(name="psTr", bufs=1, space=bass.MemorySpace.PSUM) as psTr, \
                 tc.tile_pool(name="psPP", bufs=1, space=bass.MemorySpace.PSUM) as psPP, \
                 tc.tile_pool(name="psNd", bufs=1, space=bass.MemorySpace.PSUM) as psNd:

                # node-phase PSUM bank: cols 0:256 pn1 | 256:384 pn2.
                # (start=True zeroes a whole 2KB bank's has_written bits, so
                # the cross-tile pool accumulators need their own bank.)
                ndbank = psNd.tile([128, 384], F32, tag="ndbank")
                pn1 = ndbank[:, 0:256]
                pn2 = ndbank[:, 256:384]
                accP = None  # SBUF pool accumulator, ping-ponged per tile

                for t in range(NT):
                    eftt = ep.tile([128, CW2], BF16, tag="eftt")
                    nc.sync.dma_start(eftt[:], d_eft2[:, t * CW2 : (t + 1) * CW2])
                    oht = ep.tile([128, OW], FP8, tag="oht")
                    nc.sync.dma_start(oht[:], d_oh2[:, t * OW : (t + 1) * OW])
                    nftt = ep.tile([64, 128], BF16, tag="nftt")
                    nc.sync.dma_start(nftt[:], d_nft[:, t * 128 : (t + 1) * 128])
                    pw = ep.tile([128, 4], BF16, tag="pw")
                    nc.sync.dma_start(pw[:], d_poolw[t * 128 : (t + 1) * 128, :])

                    pagg = psAgg.tile([128, 384], F32, tag="pagg")

                    for j in range(PPT):
                        pc = slice(j * 128, (j + 1) * 128)
                        bandE = psE.tile([128, 512], F32, tag="bandE")
                        bandO = psO.tile([128, 512], F32, tag="bandO")
                        pe2E = ps2E.tile([128, 128], F32, tag="pe2E")
                        pe2O = ps2O.tile([128, 128], F32, tag="pe2O")

                        # e1 edge-major (both bands run concurrently)
                        nc.tensor.matmul(bandE[:, 0:256], eftt[0:33, pc],
                                         We1T2[0:33, :], start=True, stop=True)
                        nc.tensor.matmul(bandO[:, 0:256], eftt[64:97, pc],
                                         We1T2[64:97, :], start=True, stop=True)
                        # e1T feature-major, 2 band passes
                        nc.tensor.matmul(bandE[:, 256:384], We1T2[0:33, 0:128],
                                         eftt[0:33, pc], start=True, stop=True)
                        nc.tensor.matmul(bandO[:, 384:512], We1T2[64:97, 128:256],
                                         eftt[64:97, pc], start=True, stop=True)
                        nc.tensor.matmul(bandE[:, 384:512], We1T2[0:33, 128:256],
                                         eftt[0:33, pc], start=True, stop=True)
                        nc.tensor.matmul(bandO[:, 256:384], We1T2[64:97, 0:128],
                                         eftt[64:97, pc], start=True, stop=True)
                        # globals + be2 into pe2 accumulators
                        nc.tensor.matmul(pe2E[:], eftt[32:35, pc], g2aug[32:35, :],
                                         start=True, stop=False,
                                         tile_position=(32, 0))
                        nc.tensor.matmul(pe2O[:], eftt[96:99, pc], g2aug[96:99, :],
                                         start=True, stop=False,
                                         tile_position=(96, 0))

                        # evacuate e1 (edge-major -> ef2; feature-major -> e1f8)
                        ef2 = esb.tile([128, 768], FP8, tag="ef2")
                        e1f8E = esb.tile([128, 256], FP8, tag="e1f8E")
                        e1f8O = esb.tile([128, 256], FP8, tag="e1f8O")
                        nc.scalar.activation(ef2[:, 0:256], bandE[:, 0:256], Relu)
                        nc.scalar.activation(ef2[:, 384:640], bandO[:, 0:256], Relu)
                        nc.vector.tensor_scalar_max(e1f8E[:], bandE[:, 256:512], 0.0)
                        nc.vector.tensor_scalar_max(e1f8O[:], bandO[:, 256:512], 0.0)

                        # e2 = relu(e1 @ We2.T + g) via fp8 DoubleRow
                        nc.tensor.matmul(pe2E[:], r3(e1f8E[:], 128), r3(We2DR[:], 128),
                                         start=False, stop=True, perf_mode=DR)
                        nc.tensor.matmul(pe2O[:], r3(e1f8O[:], 128), r3(We2DR[:], 128),
                                         start=False, stop=True, perf_mode=DR)
                        nc.vector.tensor_scalar_max(ef2[:, 256:384], pe2E[:], 0.0)
                        nc.vector.tensor_scalar_max(ef2[:, 640:768], pe2O[:], 0.0)

                        # segment-sum both chunks via fp8 DoubleRow one-hot
                        nc.tensor.matmul(pagg[:], r3(oht[:, j * 256 : (j + 1) * 256], 128),
                                         r3(ef2[:], 384),
                                         start=(j == 0), stop=(j == PPT - 1),
                                         perf_mode=DR)

                    # ---------------- node phase for tile t ----------------
                    aggsb = nsb.tile([128, 384], BF16, tag="aggsb")
                    nc.scalar.activation(aggsb[:], pagg[:], Copy,
                                         scale=invc2[:, t : t + 1])

                    ptr = psTr.tile([128, 384], BF16, tag="ptr")
                    nc.tensor.transpose(ptr[:, 0:128], aggsb[:, 0:128], ident[:])
                    nc.tensor.transpose(ptr[:, 128:256], aggsb[:, 128:256], ident[:])
                    nc.tensor.transpose(ptr[:, 256:384], aggsb[:, 256:384], ident[:])
                    aggT = nsb.tile([128, 384], FP8, tag="aggT")
                    nc.vector.tensor_copy(aggT[:], ptr[:])

                    # n1 (feature-major, fo-slices s=0,1)
                    for s in range(2):
                        sc = slice(s * 128, (s + 1) * 128)
                        nc.tensor.matmul(pn1[:, sc], Wn1T[:, sc], nftt[:],
                                         start=True, stop=False)
                        nc.tensor.matmul(pn1[:, sc],
                                         r3(Win1DR[:, s * 256 : (s + 1) * 256], 128),
                                         r3(aggT[:, 0:256], 128),
                                         start=False, stop=True, perf_mode=DR)
                    n1f8 = nsb.tile([128, 256], FP8, tag="n1f8")
                    for s in range(2):
                        sc = slice(s * 128, (s + 1) * 128)
                        nc.vector.tensor_scalar(
                            n1f8[:, sc], pn1[:, sc], bn1c[:, s : s + 1], 0.0,
                            op0=mybir.AluOpType.add, op1=mybir.AluOpType.max,
                        )

                    # n2 (node-major)
                    nc.tensor.matmul(pn2, nhot[:, t * 128 : (t + 1) * 128],
                                     gnaug[:], start=True, stop=False)
                    nc.tensor.matmul(pn2, r3(n1f8[:], 128), r3(Wn2DR[:], 128),
                                     start=False, stop=False, perf_mode=DR)
                    nc.tensor.matmul(pn2, aggT[:, 256:384], Win2f8[:],
                                     start=False, stop=True)
                    n2bf = nsb.tile([128, 128], BF16, tag="n2bf")
                    nc.scalar.activation(n2bf[:], pn2, Relu)

                    # pooled accumulators
                    # one long accumulation group for both pools (same bank):
                    # start only on the very first matmul, stop on the last.
                    # Per-tile pool matmuls are closed PSUM groups; the
                    # cross-tile sum lives in SBUF (ping-ponged DVE adds), so
                    # no PSUM group stays open across bank-zeroing starts.
                    ppt = psPP.tile([2, 256], F32, tag="ppt")
                    nc.tensor.matmul(ppt[:, 0:128], pw[:, 0:2], n2bf[:],
                                     start=True, stop=True)
                    nc.tensor.matmul(ppt[:, 128:256], pw[:, 2:4],
                                     aggsb[:, 256:384], start=True, stop=True)
                    accP_new = nsb.tile([2, 256], F32, tag="accP")
                    if accP is None:
                        nc.vector.tensor_copy(accP_new[:], ppt[:])
                    else:
                        nc.vector.tensor_tensor(accP_new[:], accP[:], ppt[:],
                                                op=mybir.AluOpType.add)
                    accP = accP_new

                # ----------------- final projection -----------------
                ptr2 = psTr.tile([128, 4], F32, tag="ptr")
                nc.tensor.transpose(ptr2[:, 0:2], accP[:, 0:128], ident2[:])
                nc.tensor.transpose(ptr2[:, 2:4], accP[:, 128:256], ident2[:])
                nt2 = nsb.tile([128, 4], F32, tag="nt2")
                nc.scalar.activation(nt2[:], ptr2[:], Copy)

                pouttile = psPP.tile([128, 128], F32, tag="ppt")
                pout = pouttile[:, 0:2]
                nc.tensor.matmul(pout, WgnT[:], nt2[:, 0:2], start=True, stop=False)
                nc.tensor.matmul(pout, WgeT[:], nt2[:, 2:4], start=False, stop=False)
                nc.tensor.matmul(pout, WggT[:], globT[:], start=False, stop=False)
                nc.tensor.matmul(pout, bgr[:], ones2[:], start=False, stop=True)
                outsb = nsb.tile([128, 2], F32, tag="outsb")
                nc.scalar.activation(outsb[:], pout, Copy)
                nc.sync.dma_start(d_out[:], outsb[:])

    return nc


_CACHE = {}


def _get_nc(NT, K0):
    key = (NT, K0)
    if key not in _CACHE:
        _CACHE[key] = _build(NT, K0)
    return _CACHE[key]


def _run(inputs, trace=False):
    in_maps, NT, K0 = _prepare(inputs)
    nc = _get_nc(NT, K0)
    res = run_bass_kernel_spmd(nc, in_maps, list(range(N_CORES)), trace=trace)
    out = np.zeros((N_GRAPHS, 128), np.float32)
    for c in range(N_CORES):
        r = np.asarray(res.results[c]["out"], np.float32)
        out[GPC * c] = r[:, 0]
        out[GPC * c + 1] = r[:, 1]
    return out, res


def kernel(**inputs):
    out, _ = _run(inputs, trace=False)
    return out


def kernel_traced(**inputs):
    return _run(inputs, trace=True)


# revision 22
# speedup vs baseline: 1.7983x; 1.6278x over previous
"""Trainium2 Bass kernel for a 2-layer GraphNetwork (gnn_message_passing).

Strategy (v3, "one-mode"):
  - 16 graphs across 8 cores (2/core, paired big-with-small to balance
    load); every edge's receiver is core-local, so all segment
    reductions stay on-core. [16,128] outputs gathered on host.
  - ALL matmuls run with tile_size (128,128): small contractions are
    zero-padded to 128 rows. Mode switches between row-banded / normal
    configs cost ~200-300ns on this part, so none are used; the only
    mixing is normal <-> fp8-DoubleRow, which measures ~70-90ns.
  - The e1 edge-layer matmul also produces the e2 globals/bias init in
    the same instruction (extra stationary rows: ones -> be1|be2,
    graph-one-hots -> globals projections), FD=384.
  - e2 projection (K=256) and the one-hot segment-sum over edge pairs
    (K=256) run as fp8e4 DoubleRow; one-hots are host-built fp8.
  - agg transposes are plain matmuls against an identity moving operand
    (out = lhsT.T @ I), avoiding transpose-mode switches.
  - fp32 PSUM everywhere; final projection fp32.
"""

import numpy as np
import ml_dtypes

import concourse.bass as bass
import concourse.tile as tile_mod
from concourse import tile
from concourse.bass_utils import run_bass_kernel_spmd
from concourse.vector_clock import ScopedClock

mybir = bass.mybir

N_NODES, N_EDGES, N_GRAPHS = 20000, 320000, 16
F_NODE, F_EDGE, F_GLOB = 64, 32, 16
N_CORES = 8
GPC = N_GRAPHS // N_CORES  # graphs per core = 2

BF16 = mybir.dt.bfloat16
F32 = mybir.dt.float32
FP8 = mybir.dt.float8e4
npbf16 = ml_dtypes.bfloat16
npfp8 = mybir.dt.np(FP8)
DR = mybir.MatmulPerfMode.DoubleRow

# ---------------------------------------------------------------------------
# Workaround: CoreV3 codegen rejects the TileContext final drain when it
# carries more than one semaphore wait. Split the waits across extra no-ops.
_MAX_WAITS = 1
_ENGINE_WAIT_LIMIT = 1
_SPLIT_ENGINES = None


def _split_excess_waits(nc):
    global _SPLIT_ENGINES
    if _SPLIT_ENGINES is None:
        ET = mybir.EngineType
        _SPLIT_ENGINES = {ET.PE, ET.Activation, ET.DVE, ET.SP, ET.Pool}
    ctr = [0]
    for bass_bb in nc.bb_map.values():
        bb = bass_bb.bb
        il = bb.instructions
        out = []
        changed = False
        for inst in il:
            si = inst.sync_info
            waits = list(si.on_wait) if (si and si.on_wait) else []
            if len(waits) > _ENGINE_WAIT_LIMIT and inst.engine in _SPLIT_ENGINES:
                head, keep = waits[:-_ENGINE_WAIT_LIMIT], waits[-_ENGINE_WAIT_LIMIT:]
                for i in range(0, len(head), _ENGINE_WAIT_LIMIT):
                    nop = mybir.InstNoOp(name=f"waitsplit-{ctr[0]}", ins=[], outs=[])
                    ctr[0] += 1
                    nop.engine = inst.engine
                    nop.sync_info = mybir.SyncInfo(
                        on_wait=head[i : i + _ENGINE_WAIT_LIMIT], on_update=[]
                    )
                    nc.register_instruction(nop, overwrite=True)
                    out.append(nop)
                inst.sync_info = mybir.SyncInfo(
                    on_wait=keep, on_update=list(si.on_update or [])
                )
                changed = True
            out.append(inst)
        if changed:
            bb.instructions = out


def _split_drain_and_barrier(self, tick_clock, wait_clock):
    nc = self.nc
    _split_excess_waits(nc)
    drain_inst = nc.sync.drain()
    wait_clock.add_sem_waits(
        drain_inst.ins, ScopedClock({None: tick_clock.global_clock})
    )
    mi = drain_inst.ins
    waits = list(mi.sync_info.on_wait) if (mi.sync_info and mi.sync_info.on_wait) else []
    if len(waits) > _MAX_WAITS:
        upd = list(mi.sync_info.on_update) if mi.sync_info.on_update else []
        mi.sync_info = mybir.SyncInfo(on_wait=waits[:_MAX_WAITS], on_update=upd)
        for i in range(_MAX_WAITS, len(waits), _MAX_WAITS):
            nop = nc.sync.nop(nofuse=True)
            nop.ins.sync_info = mybir.SyncInfo(
                on_wait=waits[i : i + _MAX_WAITS], on_update=[]
            )
    nc.all_engine_barrier()
    assert self.sems is not None
    popped = nc._tile_sem_poison_stack.pop()
    assert popped is self._sem_poison
    nc.clear_and_free_semaphores(list(self.sems.allocated().values()))
    nc.all_engine_barrier()


tile_mod.TileContext._drain_and_barrier = _split_drain_and_barrier


# ---------------------------------------------------------------------------
# Host-side graph partitioning / layout


def _pack_core(node_ids, degs, nt, cap_e):
    order = np.argsort(-degs, kind="stable")
    tiles_n = [[] for _ in range(nt)]
    tile_ncnt = np.zeros(nt, np.int64)
    tile_ecnt = np.zeros(nt, np.int64)
    for j in order:
        cand = np.where(tile_ncnt < 128)[0]
        if len(cand) == 0:
            return None
        t = cand[np.argmin(tile_ecnt[cand])]
        tiles_n[t].append(node_ids[j])
        tile_ncnt[t] += 1
        tile_ecnt[t] += degs[j]
    if (tile_ecnt > cap_e).any():
        return None
    return [np.array(t, dtype=np.int64) for t in tiles_n]


def _prepare(inputs):
    nf = np.asarray(inputs["node_feats"], np.float32)
    ef = np.asarray(inputs["edge_feats"], np.float32)
    glob = np.asarray(inputs["globals_"], np.float32)
    recv = np.asarray(inputs["receivers"]).astype(np.int64)
    ngraph = np.asarray(inputs["node_graph"]).astype(np.int64)

    cnt = np.bincount(recv, minlength=N_NODES).astype(np.int64)
    egraph = ngraph[recv]
    ncnt_g = np.bincount(ngraph, minlength=N_GRAPHS)
    ecnt_g = np.bincount(egraph, minlength=N_GRAPHS)

    # pair heavy graphs with light ones to balance nodes across cores
    order = np.argsort(ncnt_g, kind="stable")
    graph_core = np.zeros(N_GRAPHS, np.int64)
    graph_slot = np.zeros(N_GRAPHS, np.int64)
    core_graphs = []
    for c in range(N_CORES):
        ga, gb = int(order[c]), int(order[N_GRAPHS - 1 - c])
        graph_core[ga] = c
        graph_slot[ga] = 0
        graph_core[gb] = c
        graph_slot[gb] = 1
        core_graphs.append((ga, gb))

    node_core = graph_core[ngraph]
    edge_core = graph_core[egraph]

    core_nodes = [np.where(node_core == c)[0] for c in range(N_CORES)]
    NT = int(max((len(cn) + 127) // 128 for cn in core_nodes))

    packs = None
    K0 = max(1, int(max(np.bincount(edge_core, minlength=N_CORES)) + NT * 128 - 1)
             // (NT * 128))
    if K0 % 2:
        K0 += 1
    for k0 in range(K0, K0 + 13, 2):
        trial = []
        ok = True
        for c in range(N_CORES):
            p = _pack_core(core_nodes[c], cnt[core_nodes[c]], NT, k0 * 128)
            if p is None:
                ok = False
                break
            trial.append(p)
        if ok:
            packs, K0 = trial, k0
            break
    assert packs is not None, "bin packing failed"

    NPAD = NT * 128
    EPAD = NT * K0 * 128
    NPAIR = NT * K0 // 2

    # --- shared weights (core-independent parts)
    We1T = np.asarray(inputs["We1"], np.float32).T  # [32, 256]
    be1 = np.asarray(inputs["be1"], np.float32)
    be2 = np.asarray(inputs["be2"], np.float32)
    bn2 = np.asarray(inputs["bn2"], np.float32)

    We1TKb = np.zeros((128, 256), np.float32)
    We1TKb[0:32] = We1T
    We1TKb[32] = be1

    We2T = np.asarray(inputs["We2"], np.float32).T  # [256, 128]
    We2DR = np.concatenate([We2T[:128], We2T[128:]], axis=1)  # [128, 256]

    Wn1T = np.asarray(inputs["Wn1"], np.float32).T  # [64, 256]
    Wn1TK = np.zeros((128, 256), np.float32)
    Wn1TK[0:64] = Wn1T
    bn1c = np.asarray(inputs["bn1"], np.float32).reshape(2, 128).T.copy()  # [128,2]

    Win1T = np.asarray(inputs["Win1"], np.float32).T  # [256, 256]
    Win1DR = np.zeros((128, 512), np.float32)
    for s in range(2):
        for i in range(2):
            Win1DR[:, 256 * s + 128 * i : 256 * s + 128 * i + 128] = \
                Win1T[128 * i : 128 * i + 128, 128 * s : 128 * s + 128]

    Wn2T = np.asarray(inputs["Wn2"], np.float32).T
    Wn2DR = np.concatenate([Wn2T[:128], Wn2T[128:]], axis=1)
    Win2T = np.asarray(inputs["Win2"], np.float32).T

    Wg2T = np.asarray(inputs["Wg2"], np.float32).T  # [16, 128]
    Wng2T = np.asarray(inputs["Wng2"], np.float32).T

    w_np = {
        "We1TKb": We1TKb.astype(npbf16),
        "We2DR": We2DR.astype(npfp8),
        "Wn1TK": Wn1TK.astype(npbf16),
        "bn1c": bn1c,
        "Win1DR": Win1DR.astype(npfp8),
        "Wn2DR": Wn2DR.astype(npfp8),
        "Win2f8": Win2T.astype(npfp8),
        "WgnT": np.asarray(inputs["Wgn"], np.float32).T.copy(),
        "WgeT": np.asarray(inputs["Wge"], np.float32).T.copy(),
        "WggT": np.asarray(inputs["Wgg"], np.float32).T.copy(),
        "bgr": np.asarray(inputs["bg"], np.float32)[None, :].copy(),
        "ones2": np.ones((1, 2), np.float32),
        "ident": np.eye(128, dtype=npbf16),
        "ident2": np.eye(2, dtype=np.float32),
    }

    slot_of_node = np.full(N_NODES, -1, np.int64)
    tile_of_node = np.full(N_NODES, -1, np.int64)
    in_maps = []
    for c in range(N_CORES):
        for t in range(NT):
            ids = packs[c][t]
            slot_of_node[ids] = t * 128 + np.arange(len(ids))
            tile_of_node[ids] = t

        # ---- edges: assign slots (grouped by receiver tile)
        eidx = np.where(edge_core == c)[0]
        et = tile_of_node[recv[eidx]]
        eorder = np.argsort(et, kind="stable")
        eidx = eidx[eorder]
        et = et[eorder]
        counts = np.bincount(et, minlength=NT)
        starts = np.concatenate([[0], np.cumsum(counts)[:-1]])
        off_in = np.arange(len(eidx)) - np.repeat(starts, counts)
        dst = et * (K0 * 128) + off_in
        assert (counts <= K0 * 128).all()

        eg_loc = graph_slot[egraph[eidx]]
        # eftM: one [128,128] column-block per chunk.
        # rows 0:32 feats, 32 ones, 33 isg0, 34 isg1, rest zero.
        eftM = np.zeros((128, EPAD), np.float32)
        eftM[0:32, dst] = ef[eidx].T
        eftM[32, dst] = 1.0
        eftM[33, dst] = (eg_loc == 0)
        eftM[34, dst] = (eg_loc == 1)

        # one-hot selectors, fp8, DR pair layout
        sel = np.full(EPAD, -1, np.int64)
        sel[dst] = slot_of_node[recv[eidx]] % 128
        oh = np.zeros((EPAD, 128), np.float32)
        vmask = sel >= 0
        oh[np.where(vmask)[0], sel[vmask]] = 1.0
        oh2 = (
            oh.reshape(NPAIR, 2, 128, 128)
            .transpose(2, 0, 1, 3)
            .reshape(128, NPAIR * 256)
        )

        # merged e1 + e2-init stationary weights (per-core globals)
        ga, gb = core_graphs[c]
        gl = np.stack([glob[ga], glob[gb]])  # [2, 16]
        gp = gl @ Wg2T  # [2, 128]
        We1Kx = np.zeros((128, 384), np.float32)
        We1Kx[0:32, 0:256] = We1T
        We1Kx[32, 0:256] = be1
        We1Kx[32, 256:384] = be2
        We1Kx[33, 256:384] = gp[0]
        We1Kx[34, 256:384] = gp[1]

        gn = gl @ Wng2T
        gnaugK = np.zeros((128, 128), np.float32)
        gnaugK[0:2] = gn
        gnaugK[2] = bn2

        # ---- nodes
        slot_node = np.full(NPAD, -1, np.int64)
        for t in range(NT):
            ids = packs[c][t]
            slot_node[t * 128 : t * 128 + len(ids)] = ids
        valid = slot_node >= 0
        sn = np.where(valid, slot_node, 0)

        nftK = np.zeros((128, NPAD), np.float32)
        nftK[0:64][:, valid] = nf[sn[valid]].T

        ng_loc = graph_slot[ngraph[sn]]
        nhotK = np.zeros((128, NPAD), np.float32)
        nhotK[0] = valid * (ng_loc == 0)
        nhotK[1] = valid * (ng_loc == 1)
        nhotK[2] = valid * 1.0

        invc2 = np.zeros((NPAD, 1), np.float32)
        invc2[valid, 0] = 1.0 / np.maximum(cnt[sn[valid]], 1)
        invc2 = invc2.reshape(NT, 128).T.copy()  # [128, NT]

        # zero-padded pool weight stationaries: cols 0:2 carry the weights
        poolw2 = np.zeros((NPAD, 256), np.float32)
        for g in range(GPC):
            gid = core_graphs[c][g]
            m = valid & (ng_loc == g)
            poolw2[m, g] = 1.0 / max(ncnt_g[gid], 1)
            poolw2[m, 128 + g] = cnt[sn[m]] / max(ecnt_g[gid], 1)

        globT = gl.T.copy()  # [16, 2]

        m = {
            "eftM": eftM.astype(npbf16),
            "oh2": oh2.astype(npfp8),
            "We1Kx": We1Kx.astype(npbf16),
            "gnaugK": gnaugK.astype(npbf16),
            "nftK": nftK.astype(npbf16),
            "nhotK": nhotK.astype(npbf16),
            "invc2": invc2,
            "poolw2": poolw2.astype(npbf16),
            "globT": globT,
        }
        m.update(w_np)
        in_maps.append(m)

    return in_maps, NT, K0, [core_graphs[c] for c in range(N_CORES)]


# ---------------------------------------------------------------------------
# Device program (identical on all cores)


def _build(NT, K0):
    Relu = mybir.ActivationFunctionType.Relu
    Copy = mybir.ActivationFunctionType.Copy

    nc = bass.Bass()
    NPAD = NT * 128
    EPAD = NT * K0 * 128
    NPAIR = NT * K0 // 2
    PPT = K0 // 2  # pairs per tile
    CW = K0 * 128  # eftM cols per tile
    OW = PPT * 256  # oh2 cols per tile

    d_eftM = nc.dram_tensor("eftM", [128, EPAD], BF16, kind="ExternalInput")
    d_oh2 = nc.dram_tensor("oh2", [128, NPAIR * 256], FP8, kind="ExternalInput")
    d_We1Kx = nc.dram_tensor("We1Kx", [128, 384], BF16, kind="ExternalInput")
    d_gnaugK = nc.dram_tensor("gnaugK", [128, 128], BF16, kind="ExternalInput")
    d_nftK = nc.dram_tensor("nftK", [128, NPAD], BF16, kind="ExternalInput")
    d_nhotK = nc.dram_tensor("nhotK", [128, NPAD], BF16, kind="ExternalInput")
    d_invc2 = nc.dram_tensor("invc2", [128, NT], F32, kind="ExternalInput")
    d_poolw2 = nc.dram_tensor("poolw2", [NPAD, 256], BF16, kind="ExternalInput")
    d_globT = nc.dram_tensor("globT", [16, 2], F32, kind="ExternalInput")

    d_We1TKb = nc.dram_tensor("We1TKb", [128, 256], BF16, kind="ExternalInput")
    d_We2DR = nc.dram_tensor("We2DR", [128, 256], FP8, kind="ExternalInput")
    d_Wn1TK = nc.dram_tensor("Wn1TK", [128, 256], BF16, kind="ExternalInput")
    d_bn1c = nc.dram_tensor("bn1c", [128, 2], F32, kind="ExternalInput")
    d_Win1DR = nc.dram_tensor("Win1DR", [128, 512], FP8, kind="ExternalInput")
    d_Wn2DR = nc.dram_tensor("Wn2DR", [128, 256], FP8, kind="ExternalInput")
    d_Win2f8 = nc.dram_tensor("Win2f8", [128, 128], FP8, kind="ExternalInput")
    d_WgnT = nc.dram_tensor("WgnT", [128, 128], F32, kind="ExternalInput")
    d_WgeT = nc.dram_tensor("WgeT", [128, 128], F32, kind="ExternalInput")
    d_WggT = nc.dram_tensor("WggT", [16, 128], F32, kind="ExternalInput")
    d_bgr = nc.dram_tensor("bgr", [1, 128], F32, kind="ExternalInput")
    d_ones2 = nc.dram_tensor("ones2", [1, 2], F32, kind="ExternalInput")
    d_ident = nc.dram_tensor("ident", [128, 128], BF16, kind="ExternalInput")
    d_ident2 = nc.dram_tensor("ident2", [2, 2], F32, kind="ExternalInput")

    d_out = nc.dram_tensor("out", [128, 2], F32, kind="ExternalOutput")

    def r3(ap, blk):
        return ap.rearrange("p (a b) -> p a b", a=2, b=blk)

    with tile.TileContext(nc) as tc:
        with tc.tile_pool(name="wp", bufs=1) as wp:
            def wtile(dram, shape, dt):
                t = wp.tile(shape, dt, tag=dram.name)
                nc.sync.dma_start(t[:], dram[:])
                return t

            We1Kx = wtile(d_We1Kx, [128, 384], BF16)
            We1TKb = wtile(d_We1TKb, [128, 256], BF16)
            We2DR = wtile(d_We2DR, [128, 256], FP8)
            gnaugK = wtile(d_gnaugK, [128, 128], BF16)
            Wn1TK = wtile(d_Wn1TK, [128, 256], BF16)
            bn1c = wtile(d_bn1c, [128, 2], F32)
            Win1DR = wtile(d_Win1DR, [128, 512], FP8)
            Wn2DR = wtile(d_Wn2DR, [128, 256], FP8)
            Win2f8 = wtile(d_Win2f8, [128, 128], FP8)
            WgnT = wtile(d_WgnT, [128, 128], F32)
            WgeT = wtile(d_WgeT, [128, 128], F32)
            WggT = wtile(d_WggT, [16, 128], F32)
            bgr = wtile(d_bgr, [1, 128], F32)
            ones2 = wtile(d_ones2, [1, 2], F32)
            ident = wtile(d_ident, [128, 128], BF16)
            ident2 = wtile(d_ident2, [2, 2], F32)
            globT = wtile(d_globT, [16, 2], F32)
            invc2 = wtile(d_invc2, [128, NT], F32)

            with tc.tile_pool(name="ep", bufs=2) as ep, \
                 tc.tile_pool(name="esb", bufs=3) as esb, \
                 tc.tile_pool(name="nsb", bufs=2) as nsb, \
                 tc.tile_pool(name="psME", bufs=1, space=bass.MemorySpace.PSUM) as psME, \
                 tc.tile_pool(name="psMO", bufs=1, space=bass.MemorySpace.PSUM) as psMO, \
                 tc.tile_pool(name="psT1", bufs=2, space=bass.MemorySpace.PSUM) as psT1, \
                 tc.tile_pool(name="psAgg", bufs=1, space=bass.MemorySpace.PSUM) as psAgg, \
                 tc.tile_pool(name="psTr", bufs=1, space=bass.MemorySpace.PSUM) as psTr, \
                 tc.tile_pool(name="psPP", bufs=1, space=bass.MemorySpace.PSUM) as psPP, \
                 tc.tile_pool(name="psNd", bufs=1, space=bass.MemorySpace.PSUM) as psNd:

                ndbank = psNd.tile([128, 384], F32, tag="ndbank")
                pn1 = ndbank[:, 0:256]
                pn2 = ndbank[:, 256:384]
                accP = None

                for t in range(NT):
                    eftt = ep.tile([128, CW], BF16, tag="eftt")
                    nc.sync.dma_start(eftt[:], d_eftM[:, t * CW : (t + 1) * CW])
                    oht = ep.tile([128, OW], FP8, tag="oht")
                    nc.sync.dma_start(oht[:], d_oh2[:, t * OW : (t + 1) * OW])
                    nftt = ep.tile([128, 128], BF16, tag="nftt")
                    nc.sync.dma_start(nftt[:], d_nftK[:, t * 128 : (t + 1) * 128])
                    nht = ep.tile([128, 128], BF16, tag="nht")
                    nc.sync.dma_start(nht[:], d_nhotK[:, t * 128 : (t + 1) * 128])
                    pw = ep.tile([128, 256], BF16, tag="pw")
                    nc.sync.dma_start(pw[:], d_poolw2[t * 128 : (t + 1) * 128, :])

                    pagg = psAgg.tile([128, 384], F32, tag="pagg")

                    for j in range(PPT):
                        e0 = slice(2 * j * 128, 2 * j * 128 + 128)
                        e1s = slice((2 * j + 1) * 128, (2 * j + 1) * 128 + 128)
                        epr = slice(2 * j * 128, 2 * j * 128 + 256)

                        mgE = psME.tile([128, 384], F32, tag="mgE")
                        mgO = psMO.tile([128, 384], F32, tag="mgO")
                        e1T2 = psT1.tile([128, 512], F32, tag="e1T2")

                        # merged e1 + e2-init (FD=384), one per chunk
                        nc.tensor.matmul(mgE[:], eftt[:, e0], We1Kx[:],
                                         start=True, stop=False)
                        nc.tensor.matmul(mgO[:], eftt[:, e1s], We1Kx[:],
                                         start=True, stop=False)
                        # e1T blocks for the pair (FD=256 each)
                        nc.tensor.matmul(e1T2[:, 0:256], We1TKb[:, 0:128],
                                         eftt[:, epr], start=True, stop=True)
                        nc.tensor.matmul(e1T2[:, 256:512], We1TKb[:, 128:256],
                                         eftt[:, epr], start=True, stop=True)

                        # evacuate e1 feat-major -> e1f8 (DVE), reshuffling
                        # blk-major -> chunk-major while casting:
                        # src cols (blk,chunk,e) b*256+c*128+e -> dst c*256+b*128+e
                        ef2 = esb.tile([128, 768], FP8, tag="ef2")
                        e1f8 = esb.tile([128, 512], FP8, tag="e1f8")
                        src = e1T2[:].rearrange("p (b c e) -> p c b e", b=2, c=2, e=128)
                        dst = e1f8[:].rearrange("p (c b e) -> p c b e", c=2, b=2, e=128)
                        nc.vector.tensor_scalar_max(dst, src, 0.0)

                        # e2 = relu(e1 @ We2.T + init) via fp8 DR
                        nc.tensor.matmul(mgE[:, 256:384], r3(e1f8[:, 0:256], 128),
                                         r3(We2DR[:], 128),
                                         start=False, stop=True, perf_mode=DR)
                        nc.tensor.matmul(mgO[:, 256:384], r3(e1f8[:, 256:512], 128),
                                         r3(We2DR[:], 128),
                                         start=False, stop=True, perf_mode=DR)
                        # evacuations (groups closed by the DRs above)
                        nc.scalar.activation(ef2[:, 0:256], mgE[:, 0:256], Relu)
                        nc.scalar.activation(ef2[:, 384:640], mgO[:, 0:256], Relu)
                        nc.vector.tensor_scalar_max(ef2[:, 256:384], mgE[:, 256:384], 0.0)
                        nc.vector.tensor_scalar_max(ef2[:, 640:768], mgO[:, 256:384], 0.0)

                        # segment-sum the pair via fp8 DR one-hot
                        nc.tensor.matmul(pagg[:], r3(oht[:, j * 256 : (j + 1) * 256], 128),
                                         r3(ef2[:], 384),
                                         start=(j == 0), stop=(j == PPT - 1),
                                         perf_mode=DR)

                    # ---------------- node phase for tile t ----------------
                    aggsb = nsb.tile([128, 384], BF16, tag="aggsb")
                    nc.scalar.activation(aggsb[:], pagg[:], Copy,
                                         scale=invc2[:, t : t + 1])

                    # transposes as normal matmuls: out = aggsb_blk.T @ I
                    ptr = psTr.tile([128, 384], F32, tag="ptr")
                    nc.tensor.matmul(ptr[:, 0:128], aggsb[:, 0:128], ident[:],
                                     start=True, stop=True)
                    nc.tensor.matmul(ptr[:, 128:256], aggsb[:, 128:256], ident[:],
                                     start=True, stop=True)
                    nc.tensor.matmul(ptr[:, 256:384], aggsb[:, 256:384], ident[:],
                                     start=True, stop=True)
                    aggT = nsb.tile([128, 384], FP8, tag="aggT")
                    nc.vector.tensor_copy(aggT[:], ptr[:])

                    # n1 (feature-major, fo-slices s=0,1)
                    for s in range(2):
                        sc = slice(s * 128, (s + 1) * 128)
                        nc.tensor.matmul(pn1[:, sc], Wn1TK[:, sc], nftt[:],
                                         start=True, stop=False)
                        nc.tensor.matmul(pn1[:, sc],
                                         r3(Win1DR[:, s * 256 : (s + 1) * 256], 128),
                                         r3(aggT[:, 0:256], 128),
                                         start=False, stop=True, perf_mode=DR)
                    n1f8 = nsb.tile([128, 256], FP8, tag="n1f8")
                    for s in range(2):
                        sc = slice(s * 128, (s + 1) * 128)
                        nc.vector.tensor_scalar(
                            n1f8[:, sc], pn1[:, sc], bn1c[:, s : s + 1], 0.0,
                            op0=mybir.AluOpType.add, op1=mybir.AluOpType.max,
                        )

                    # n2 (node-major)
                    nc.tensor.matmul(pn2, nht[:], gnaugK[:], start=True, stop=False)
                    nc.tensor.matmul(pn2, r3(n1f8[:], 128), r3(Wn2DR[:], 128),
                                     start=False, stop=False, perf_mode=DR)
                    nc.tensor.matmul(pn2, aggT[:, 256:384], Win2f8[:],
                                     start=False, stop=True)
                    n2bf = nsb.tile([128, 128], BF16, tag="n2bf")
                    nc.scalar.activation(n2bf[:], pn2, Relu)

                    # pools: per-tile closed groups, accumulated in SBUF
                    ppt = psPP.tile([128, 256], F32, tag="ppt")
                    nc.tensor.matmul(ppt[:, 0:128], pw[:, 0:128], n2bf[:],
                                     start=True, stop=True)
                    nc.tensor.matmul(ppt[:, 128:256], pw[:, 128:256],
                                     aggsb[:, 256:384], start=True, stop=True)
                    accP_new = nsb.tile([2, 256], F32, tag="accP")
                    if accP is None:
                        nc.vector.tensor_copy(accP_new[:], ppt[0:2, :])
                    else:
                        nc.vector.tensor_tensor(accP_new[:], accP[:], ppt[0:2, :],
                                                op=mybir.AluOpType.add)
                    accP = accP_new

                # ----------------- final projection -----------------
                ptr2 = psTr.tile([128, 4], F32, tag="ptr")
                nc.tensor.matmul(ptr2[:, 0:2], accP[:, 0:128], ident2[:],
                                 start=True, stop=True)
                nc.tensor.matmul(ptr2[:, 2:4], accP[:, 128:256], ident2[:],
                                 start=True, stop=True)
                nt2 = nsb.tile([128, 4], F32, tag="nt2")
                nc.scalar.activation(nt2[:], ptr2[:], Copy)

                pouttile = psPP.tile([128, 256], F32, tag="ppt")
                pout = pouttile[:, 0:2]
                nc.tensor.matmul(pout, WgnT[:], nt2[:, 0:2], start=True, stop=False)
                nc.tensor.matmul(pout, WgeT[:], nt2[:, 2:4], start=False, stop=False)
                nc.tensor.matmul(pout, WggT[:], globT[:], start=False, stop=False)
                nc.tensor.matmul(pout, bgr[:], ones2[:], start=False, stop=True)
                outsb = nsb.tile([128, 2], F32, tag="outsb")
                nc.scalar.activation(outsb[:], pout, Copy)
                nc.sync.dma_start(d_out[:], outsb[:])

    return nc


_CACHE = {}


def _get_nc(NT, K0):
    key = (NT, K0)
    if key not in _CACHE:
        _CACHE[key] = _build(NT, K0)
    return _CACHE[key]


def _run(inputs, trace=False):
    in_maps, NT, K0, core_graphs = _prepare(inputs)
    nc = _get_nc(NT, K0)
    res = run_bass_kernel_spmd(nc, in_maps, list(range(N_CORES)), trace=trace)
    out = np.zeros((N_GRAPHS, 128), np.float32)
    for c in range(N_CORES):
        r = np.asarray(res.results[c]["out"], np.float32)
        ga, gb = core_graphs[c]
        out[ga] = r[:, 0]
        out[gb] = r[:, 1]
    return out, res


def kernel(**inputs):
    out, _ = _run(inputs, trace=False)
    return out


def kernel_traced(**inputs):
    return _run(inputs, trace=True)
